# revision 16
# baseline (speedup 1.0000x reference)
# Trainium2 Bass kernel for nn_AxialAttention (8 NeuronCores, W-parallel).
#
# Sharding: the W axis (axis=2, the vmapped axis) is split into 8 contiguous
# slices of 32 columns, one per core. Every part of the computation (the four
# 1x1-conv GEMMs, the per-(head, w) axial attention, the embedding terms) is
# independent across w, so there are no collectives; the small weight matrices
# and embedding tables are replicated to every core.
#
# Per-core math for one w column (all heads):
#   qsT[x, (h c)] = query[:, :, w].T @ Wq.T     (fp8 DoubleRow, K=256/pass)
#   khT[x, (h c)] = key_[:, :, w].T @ Wk.T      (fp8 DoubleRow)
#   vh [(h c), x] = Wv @ value[:, :, w]         (bf16)
#   logits_h[C, c] = khT_h.T @ qsT_h + qe.T @ qsT + ke.T @ khT
#     (the emb terms are full-width fp8 DoubleRow matmuls over the dup'd
#      tables; scales: qin/kin fp8 at 1x, Wq/Wk fp8 at 64x, qsT/khT stored
#      fp8 at 16x -> logits accumulate 4096*(true logits); softmax's
#      1/sqrt(256) is part of that 4096)
#   E = exp(logits / 4096)        (max-subtraction unnecessary: |logits|<~2)
#   U_h = E_h.T @ [vh_h + ve | 1]          (ones column gives the softmax
#   attn_h = U_h[:, :256] / U_h[:, 256]     denominator for free)
#   out[:, :, w] = Wo @ attn                (bf16)
#
# fp8 (e4m3) is used only where quantization noise lands pre-softmax (the
# q/k path): measured 9.6e-3 max-rel on the final output vs the 2e-2 gate.
# The v/o path stays bf16 - fp8 there pushes the error to 3.5e-2.
# DoubleRow packs two fp8 weights per PE cell, contracting K=256 per pass:
# the q/k projection matmul count halves (~1.75x on those GEMMs).
#
# Heads are packed even/odd into the two 64-partition halves so the per-head
# 64x64 logits matmuls and the 64-row attention matmuls run as concurrent
# PE row/column tiles (tile_position diagonal packing). Large PSUM->SBUF
# evacuations are split across the Scalar/Vector/GpSimd engines to keep
# every engine under the PE's per-pair budget.

import numpy as np

H = 8          # heads
QK = 64        # per-head qk/vo channels
C = 512        # io channels
X = 256        # spatial H (attention contraction axis)
W = 256        # spatial W (vmapped axis, sharded)
N_CORES = 8
WC = W // N_CORES   # w columns per core
PAIRS = WC // 2

_CACHE = {}


def _build_program():
    import concourse.mybir as mybir
    import concourse.tile as tile
    from concourse import bacc

    f32 = mybir.dt.float32
    bf16 = mybir.dt.bfloat16
    fp8 = mybir.dt.float8e4
    AF = mybir.ActivationFunctionType
    DR = mybir.MatmulPerfMode.DoubleRow

    nc = bacc.Bacc("TRN2", target_bir_lowering=False, debug=False,
                   num_devices=N_CORES)

    qin = nc.dram_tensor("qin", [PAIRS, C, 2, X], fp8, kind="ExternalInput").ap()
    kin = nc.dram_tensor("kin", [PAIRS, C, 2, X], fp8, kind="ExternalInput").ap()
    vin = nc.dram_tensor("vin", [PAIRS, C, 2, X], bf16, kind="ExternalInput").ap()
    wqt = nc.dram_tensor("wqt", [C, C], fp8, kind="ExternalInput").ap()
    wkt = nc.dram_tensor("wkt", [C, C], fp8, kind="ExternalInput").ap()
    wvt = nc.dram_tensor("wvt", [C, C], bf16, kind="ExternalInput").ap()
    wot = nc.dram_tensor("wot", [C, C], bf16, kind="ExternalInput").ap()
    qe2 = nc.dram_tensor("qe2", [X, 2 * QK], fp8, kind="ExternalInput").ap()
    ke2 = nc.dram_tensor("ke2", [X, 2 * QK], fp8, kind="ExternalInput").ap()
    vet = nc.dram_tensor("vet", [QK, X], f32, kind="ExternalInput").ap()
    out = nc.dram_tensor("out", [C, WC, X], f32, kind="ExternalOutput").ap()

    KT = C // 128   # 4 contraction tiles of the channel dim
    XT = X // 128   # 2 tiles of the spatial-x dim

    with tile.TileContext(nc) as tc:
        with (
            tc.tile_pool(name="consts", bufs=1) as consts,
            tc.tile_pool(name="inp", bufs=4) as inp,
            tc.tile_pool(name="qkt", bufs=2) as qkt,
            tc.tile_pool(name="mid", bufs=2) as mid,
            tc.tile_pool(name="small", bufs=8) as small,
            tc.tile_pool(name="psA", bufs=3, space="PSUM") as psA,
            tc.tile_pool(name="psVL", bufs=2, space="PSUM") as psVL,
            tc.tile_pool(name="psU", bufs=3, space="PSUM") as psU,
        ):
            def load_inputs(pair):
                q_t = inp.tile([128, KT, 2, X], fp8, tag="q_t")
                nc.sync.dma_start(
                    q_t[:], qin[pair].rearrange("(kt p) w x -> p kt (w x)", p=128))
                k_t = inp.tile([128, KT, 2, X], fp8, tag="k_t")
                nc.sync.dma_start(
                    k_t[:], kin[pair].rearrange("(kt p) w x -> p kt (w x)", p=128))
                v_t = inp.tile([128, KT, 2, X], bf16, tag="v_t")
                nc.sync.dma_start(
                    v_t[:], vin[pair].rearrange("(kt p) w x -> p kt (w x)", p=128))
                return q_t, k_t, v_t

            # pair-0 inputs first so the PE can start ASAP; q is split per
            # k-tile so the first matmul only waits for one chunk. Constants
            # go on the ACT HWDGE ring so the two DMA streams run in parallel.
            q0 = inp.tile([128, KT, 2, X], fp8, tag="q_t")
            for kt in range(KT):
                nc.sync.dma_start(
                    q0[:, kt, :, :],
                    qin[0, kt * 128:(kt + 1) * 128].rearrange("p w x -> p (w x)"))
            k0 = inp.tile([128, KT, 2, X], fp8, tag="k_t")
            nc.sync.dma_start(
                k0[:], kin[0].rearrange("(kt p) w x -> p kt (w x)", p=128))
            v0 = inp.tile([128, KT, 2, X], bf16, tag="v_t")
            nc.sync.dma_start(
                v0[:], vin[0].rearrange("(kt p) w x -> p kt (w x)", p=128))
            prefetched = (q0, k0, v0)

            # wq first (the first matmul needs only its kt 0-1 half), then wk.
            wq_sb = consts.tile([128, KT, C], fp8)
            wqr = wqt.rearrange("(kt p) o -> p kt o", p=128)
            nc.scalar.dma_start(wq_sb[:, 0:2, :], wqr[:, 0:2, :])
            nc.scalar.dma_start(wq_sb[:, 2:4, :], wqr[:, 2:4, :])
            wk_sb = consts.tile([128, KT, C], fp8)
            nc.scalar.dma_start(wk_sb[:], wkt.rearrange("(kt p) o -> p kt o", p=128))
            wv_sb = consts.tile([128, KT, C], bf16)
            nc.scalar.dma_start(wv_sb[:], wvt.rearrange("(kt p) o -> p kt o", p=128))
            wo_sb = consts.tile([128, KT, C], bf16)
            nc.scalar.dma_start(wo_sb[:], wot.rearrange("(kt p) o -> p kt o", p=128))
            qe_sb = consts.tile([128, XT, 2 * QK], fp8)
            nc.scalar.dma_start(qe_sb[:], qe2.rearrange("(xt p) m -> p xt m", p=128))
            ke_sb = consts.tile([128, XT, 2 * QK], fp8)
            nc.scalar.dma_start(ke_sb[:], ke2.rearrange("(xt p) m -> p xt m", p=128))
            ve_sb = consts.tile([128, X], f32)
            nc.scalar.dma_start(ve_sb[0:QK, :], vet[:])
            nc.scalar.dma_start(ve_sb[QK:128, :], vet[:])

            # vplus double-buffer with the ones columns filled exactly once
            # (they never change; pool rotation would clobber them).
            vplus_bufs = []
            for b in range(2):
                vb = mid.tile([128, KT, 2, X + 2], bf16, tag=f"vplus{b}")
                nc.vector.memset(vb[:, :, :, X:X + 2], 1.0)
                vplus_bufs.append(vb)

            # Software pipeline, 2 pairs deep. The PE queue is FIFO, so the
            # exp -> pu and attn-evac -> o-GEMM waits of pair i are hidden
            # behind pair i+1's projection matmuls:
            #   iter i issues: proj(i), v(i), logits+exp(i), pu(i-1), o(i-2)
            e_hist = {}     # pair -> e_t tile (exp of logits, both wi)
            attn_hist = {}  # pair -> attn tile

            for it in range(PAIRS + 2):
                if it < PAIRS:
                    q_t, k_t, v_t = prefetched if it == 0 else load_inputs(it)

                    # --- q/k projections (fp8 DoubleRow), transposed ---
                    # qsT/khT [x, (h c)] stored fp8 at 16x the projection.
                    qsT = qkt.tile([128, 2, XT, C], fp8)   # [x_p, w, xt, o]
                    khT = qkt.tile([128, 2, XT, C], fp8)
                    for wi in range(2):
                        for xt in range(XT):
                            pq = psA.tile([128, C], f32, tag="mm")
                            for kp in range(KT // 2):
                                nc.tensor.matmul(
                                    pq[:],
                                    q_t[:, 2 * kp:2 * kp + 2, wi,
                                        xt * 128:(xt + 1) * 128],
                                    wq_sb[:, 2 * kp:2 * kp + 2, :],
                                    start=(kp == 0), stop=(kp == KT // 2 - 1),
                                    perf_mode=DR)
                            nc.scalar.activation(qsT[:, wi, xt, :], pq[:],
                                                 AF.Copy, scale=0.25)
                            pk = psA.tile([128, C], f32, tag="mm")
                            for kp in range(KT // 2):
                                nc.tensor.matmul(
                                    pk[:],
                                    k_t[:, 2 * kp:2 * kp + 2, wi,
                                        xt * 128:(xt + 1) * 128],
                                    wk_sb[:, 2 * kp:2 * kp + 2, :],
                                    start=(kp == 0), stop=(kp == KT // 2 - 1),
                                    perf_mode=DR)
                            nc.vector.tensor_scalar_mul(khT[:, wi, xt, :],
                                                        pk[:], 0.25)

                    # --- v projection + ve add (bf16) ---
                    vplus = vplus_bufs[it % 2]   # [c2_p, head-pair, w, x+2]
                    for ot in range(KT):
                        pv = psVL.tile([128, 2, X], f32, tag="vl")
                        for kt in range(KT):
                            nc.tensor.matmul(
                                pv[:],
                                wv_sb[:, kt, ot * 128:(ot + 1) * 128],
                                v_t[:, kt, :, :],
                                start=(kt == 0), stop=(kt == KT - 1))
                        for wi in range(2):
                            nc.vector.tensor_add(
                                vplus[:, ot, wi, 0:X], pv[:, wi, :], ve_sb[:])

                    # --- logits + exp (consumed by pu next iteration) ---
                    e_t = mid.tile([128, 2, C], bf16, tag="e")
                    for wi in range(2):
                        pl = psVL.tile([128, C], f32, tag="vl")
                        # k_emb and q_emb terms, all heads at once (dup'd
                        # tables; plain matmuls per x tile)
                        nc.tensor.matmul(pl[:], ke_sb[:, 0, :], khT[:, wi, 0, :],
                                         start=True, stop=False)
                        nc.tensor.matmul(pl[:], ke_sb[:, 1, :], khT[:, wi, 1, :],
                                         start=False, stop=False)
                        nc.tensor.matmul(pl[:], qe_sb[:, 0, :], qsT[:, wi, 0, :],
                                         start=False, stop=False)
                        nc.tensor.matmul(pl[:], qe_sb[:, 1, :], qsT[:, wi, 1, :],
                                         start=False, stop=False)
                        # per-head khT^T @ qsT term. The C-half each head's
                        # logits occupy alternates per head-pair (swap for odd
                        # pairs) so the pu matmuls can use all 4 PE quadrants.
                        for h in range(H):
                            half = ((h % 2) ^ ((h // 2) % 2)) * QK
                            cb = h * QK
                            for xt in range(XT):
                                nc.tensor.matmul(
                                    pl[half:half + QK, cb:cb + QK],
                                    khT[:, wi, xt, cb:cb + QK],
                                    qsT[:, wi, xt, cb:cb + QK],
                                    start=False,
                                    stop=(h == H - 1 and xt == XT - 1),
                                    tile_position=(0, half))
                        nc.scalar.activation(e_t[:, wi, :], pl[:], AF.Exp,
                                             scale=1.0 / 4096.0)
                    e_hist[it] = e_t

                if 1 <= it <= PAIRS:
                    # --- attention weights x values for pair it-1 ---
                    pr = it - 1
                    e_t = e_hist.pop(pr)
                    vplus = vplus_bufs[pr % 2]
                    attn = mid.tile([128, KT, 2, X], bf16)  # [(h c)_p, kt, w, x]
                    for wi in range(2):
                        for tp in range(0, KT, 2):   # two head pairs at a time
                            pus = []
                            for t in (tp, tp + 1):
                                pu = psU.tile([128, X + 2], f32, tag="pu")
                                for j in range(2):
                                    h = 2 * t + j
                                    row = ((j ^ (t % 2)) * QK)  # e_t C-half
                                    col = j * QK                # attn layout
                                    nc.tensor.matmul(
                                        pu[col:col + QK, :],
                                        e_t[row:row + QK, wi,
                                            h * QK:(h + 1) * QK],
                                        vplus[row:row + QK, t, wi, :],
                                        start=True, stop=True,
                                        tile_position=(row, col))
                                pus.append(pu)
                            for t, pu in zip((tp, tp + 1), pus):
                                recip = small.tile([128, 1], f32, tag="recip")
                                nc.vector.reciprocal(recip[:], pu[:, X:X + 1])
                                if t != 3:   # scalar is lighter-loaded
                                    nc.scalar.activation(
                                        attn[:, t, wi, :],
                                        pu[:, 0:X], AF.Copy, scale=recip[:])
                                else:
                                    nc.vector.tensor_scalar_mul(
                                        attn[:, t, wi, :], pu[:, 0:X],
                                        recip[:])
                    attn_hist[pr] = attn

                if it >= 2:
                    # --- output projection for pair it-2 ---
                    pr = it - 2
                    w0 = pr * 2
                    attn = attn_hist.pop(pr)
                    for ot in range(KT):
                        po = psA.tile([128, 2, X], f32, tag="mm")
                        for kt in range(KT):
                            nc.tensor.matmul(
                                po[:],
                                wo_sb[:, kt, ot * 128:(ot + 1) * 128],
                                attn[:, kt, :, :],
                                start=(kt == 0), stop=(kt == KT - 1))
                        ob = small.tile([128, 2, X], f32, tag="ob")
                        nc.scalar.activation(ob[:, 0, :], po[:, 0, :], AF.Copy)
                        nc.vector.tensor_copy(ob[:, 1, :], po[:, 1, :])
                        # scalar HWDGE ring: idle after the constant loads, and
                        # keeps output traffic off the input (sync) ring
                        nc.scalar.dma_start(
                            out[ot * 128:(ot + 1) * 128, w0:w0 + 2, :], ob[:])

    nc.compile()
    return nc


def _get_program():
    if "nc" not in _CACHE:
        _CACHE["nc"] = _build_program()
    return _CACHE["nc"]


def _make_in_maps(query, key_, value, Wq, Wk, Wv, Wo, q_emb, k_emb, v_emb):
    import ml_dtypes
    bf16 = ml_dtypes.bfloat16
    fp8 = ml_dtypes.float8_e4m3

    def q8(a, scale):
        return np.ascontiguousarray(
            np.clip(a * np.float32(scale), -240, 240).astype(fp8))

    # Scale plan (logits accumulate 4096x, undone in the exp activation):
    #   qin/kin fp8 at 1x; Wq/Wk fp8 at 64x -> psum 64x; evac scale 0.25
    #   -> qsT/khT fp8 at 16x; per-head term 256x = 4096 * (1/16 softmax).
    #   q_emb fp8 at 16x (pairs with qsT); k_emb fp8 at 256x (pairs with khT,
    #   no softmax scale on the k.ke term).
    wqt = q8(Wq.T, 64.0)
    wkt = q8(Wk.T, 64.0)
    # Swap the two 64-row head blocks inside odd 128-blocks of Wv's output
    # dim: odd head-pairs keep their even/odd heads on swapped partition
    # halves so the pu matmuls can occupy all four PE quadrants.
    vperm = np.arange(C).reshape(C // 128, 2, QK)[:, [0, 1], :].copy()
    vperm[1::2] = vperm[1::2][:, [1, 0], :]
    wvt = np.ascontiguousarray(Wv[vperm.reshape(-1)].T.astype(bf16))
    wot = np.ascontiguousarray(Wo.T.astype(bf16))
    qe2 = q8(np.concatenate([q_emb, q_emb], axis=1), 16.0)
    ke2 = q8(np.concatenate([k_emb, k_emb], axis=1), 256.0)
    vet = np.ascontiguousarray(v_emb.T)

    def shard8(a, ws):
        # (C, X, WC) -> [pair, c, w, x] contiguous fp8
        return q8(
            a[:, :, ws].reshape(C, X, PAIRS, 2).transpose(2, 0, 3, 1), 1.0)

    def shardb(a, ws):
        return np.ascontiguousarray(
            a[:, :, ws].reshape(C, X, PAIRS, 2).transpose(2, 0, 3, 1).astype(bf16))

    in_maps = []
    for c in range(N_CORES):
        ws = slice(c * WC, (c + 1) * WC)
        in_maps.append({
            "qin": shard8(query, ws),
            "kin": shard8(key_, ws),
            "vin": shardb(value, ws),
            "wqt": wqt, "wkt": wkt, "wvt": wvt, "wot": wot,
            "qe2": qe2, "ke2": ke2, "vet": vet,
        })
    return in_maps


def _run(in_maps, trace=False):
    from concourse.bass_utils import run_bass_kernel_spmd
    nc = _get_program()
    return run_bass_kernel_spmd(nc, in_maps, list(range(N_CORES)), trace=trace)


def kernel(query, key_, value, Wq, Wk, Wv, Wo, q_emb, k_emb, v_emb):
    args = (query, key_, value, Wq, Wk, Wv, Wo, q_emb, k_emb, v_emb)
    in_maps = _make_in_maps(*[np.ascontiguousarray(a, np.float32) for a in args])
    res = _run(in_maps, trace=False)
    out = np.empty((C, X, W), np.float32)
    for c in range(N_CORES):
        out[:, :, c * WC:(c + 1) * WC] = res.results[c]["out"].transpose(0, 2, 1)
    return out


# revision 17
# speedup vs baseline: 1.0694x; 1.0694x over previous
# Trainium2 Bass kernel for nn_AxialAttention (8 NeuronCores, W-parallel).
#
# Sharding: the W axis (axis=2, the vmapped axis) is split into 8 contiguous
# slices of 32 columns, one per core. Every part of the computation (the four
# 1x1-conv GEMMs, the per-(head, w) axial attention, the embedding terms) is
# independent across w, so there are no collectives; the small weight matrices
# and embedding tables are replicated to every core.
#
# Per-core math for one w column (all heads):
#   qsT[x, (h c)] = query[:, :, w].T @ Wq.T     (fp8 DoubleRow, K=256/pass)
#   khT[x, (h c)] = key_[:, :, w].T @ Wk.T      (fp8 DoubleRow)
#   vh [(h c), x] = Wv @ value[:, :, w]         (bf16)
#   logits_h[C, c] = khT_h.T @ qsT_h + qe.T @ qsT + ke.T @ khT
#     (the emb terms are full-width fp8 DoubleRow matmuls over the dup'd
#      tables; scales: qin/kin fp8 at 1x, Wq/Wk fp8 at 64x, qsT/khT stored
#      fp8 at 16x -> logits accumulate 4096*(true logits); softmax's
#      1/sqrt(256) is part of that 4096)
#   E = exp(logits / 4096)        (max-subtraction unnecessary: |logits|<~2)
#   U_h = E_h.T @ [vh_h + ve | 1]          (ones column gives the softmax
#   attn_h = U_h[:, :256] / U_h[:, 256]     denominator for free)
#   out[:, :, w] = Wo @ attn                (bf16)
#
# fp8 (e4m3) is used only where quantization noise lands pre-softmax (the
# q/k path): measured 9.6e-3 max-rel on the final output vs the 2e-2 gate.
# The v/o path stays bf16 - fp8 there pushes the error to 3.5e-2.
# DoubleRow packs two fp8 weights per PE cell, contracting K=256 per pass:
# the q/k projection matmul count halves (~1.75x on those GEMMs).
#
# Heads are packed even/odd into the two 64-partition halves so the per-head
# 64x64 logits matmuls and the 64-row attention matmuls run as concurrent
# PE row/column tiles (tile_position diagonal packing). Large PSUM->SBUF
# evacuations are split across the Scalar/Vector/GpSimd engines to keep
# every engine under the PE's per-pair budget.

import numpy as np

H = 8          # heads
QK = 64        # per-head qk/vo channels
C = 512        # io channels
X = 256        # spatial H (attention contraction axis)
W = 256        # spatial W (vmapped axis, sharded)
N_CORES = 8
WC = W // N_CORES   # w columns per core
PAIRS = WC // 2

_CACHE = {}


def _build_program():
    import concourse.mybir as mybir
    import concourse.tile as tile
    from concourse import bacc

    f32 = mybir.dt.float32
    bf16 = mybir.dt.bfloat16
    fp8 = mybir.dt.float8e4
    AF = mybir.ActivationFunctionType
    DR = mybir.MatmulPerfMode.DoubleRow

    nc = bacc.Bacc("TRN2", target_bir_lowering=False, debug=False,
                   num_devices=N_CORES)

    qin = nc.dram_tensor("qin", [PAIRS, C, 2, X], fp8, kind="ExternalInput").ap()
    kin = nc.dram_tensor("kin", [PAIRS, C, 2, X], fp8, kind="ExternalInput").ap()
    vin = nc.dram_tensor("vin", [PAIRS, C, 2, X], bf16, kind="ExternalInput").ap()
    wqt = nc.dram_tensor("wqt", [C, C], fp8, kind="ExternalInput").ap()
    wkt = nc.dram_tensor("wkt", [C, C], fp8, kind="ExternalInput").ap()
    wvt = nc.dram_tensor("wvt", [C, C], bf16, kind="ExternalInput").ap()
    wot = nc.dram_tensor("wot", [C, C], bf16, kind="ExternalInput").ap()
    qe2 = nc.dram_tensor("qe2", [X, 2 * QK], fp8, kind="ExternalInput").ap()
    ke2 = nc.dram_tensor("ke2", [X, 2 * QK], fp8, kind="ExternalInput").ap()
    vet = nc.dram_tensor("vet", [QK, X], f32, kind="ExternalInput").ap()
    out = nc.dram_tensor("out", [C, WC, X], f32, kind="ExternalOutput").ap()

    KT = C // 128   # 4 contraction tiles of the channel dim
    XT = X // 128   # 2 tiles of the spatial-x dim

    with tile.TileContext(nc) as tc:
        with (
            tc.tile_pool(name="consts", bufs=1) as consts,
            tc.tile_pool(name="inp", bufs=4) as inp,
            tc.tile_pool(name="qkt", bufs=2) as qkt,
            tc.tile_pool(name="mid", bufs=2) as mid,
            tc.tile_pool(name="small", bufs=8) as small,
            tc.tile_pool(name="psA", bufs=3, space="PSUM") as psA,
            tc.tile_pool(name="psVL", bufs=2, space="PSUM") as psVL,
            tc.tile_pool(name="psU", bufs=3, space="PSUM") as psU,
        ):
            def load_inputs(pair):
                q_t = inp.tile([128, KT, 2, X], fp8, tag="q_t")
                nc.sync.dma_start(
                    q_t[:], qin[pair].rearrange("(kt p) w x -> p kt (w x)", p=128))
                k_t = inp.tile([128, KT, 2, X], fp8, tag="k_t")
                nc.sync.dma_start(
                    k_t[:], kin[pair].rearrange("(kt p) w x -> p kt (w x)", p=128))
                v_t = inp.tile([128, KT, 2, X], bf16, tag="v_t")
                nc.sync.dma_start(
                    v_t[:], vin[pair].rearrange("(kt p) w x -> p kt (w x)", p=128))
                return q_t, k_t, v_t

            # pair-0 inputs first so the PE can start ASAP; q is split per
            # k-tile so the first matmul only waits for one chunk. Constants
            # go on the ACT HWDGE ring so the two DMA streams run in parallel.
            q0 = inp.tile([128, KT, 2, X], fp8, tag="q_t")
            for kt in range(KT):
                nc.sync.dma_start(
                    q0[:, kt, :, :],
                    qin[0, kt * 128:(kt + 1) * 128].rearrange("p w x -> p (w x)"))
            k0 = inp.tile([128, KT, 2, X], fp8, tag="k_t")
            nc.sync.dma_start(
                k0[:], kin[0].rearrange("(kt p) w x -> p kt (w x)", p=128))
            v0 = inp.tile([128, KT, 2, X], bf16, tag="v_t")
            nc.sync.dma_start(
                v0[:], vin[0].rearrange("(kt p) w x -> p kt (w x)", p=128))
            prefetched = (q0, k0, v0)

            # wq first (the first matmul needs only its kt 0-1 half), then wk.
            wq_sb = consts.tile([128, KT, C], fp8)
            wqr = wqt.rearrange("(kt p) o -> p kt o", p=128)
            nc.scalar.dma_start(wq_sb[:, 0:2, :], wqr[:, 0:2, :])
            nc.scalar.dma_start(wq_sb[:, 2:4, :], wqr[:, 2:4, :])
            wk_sb = consts.tile([128, KT, C], fp8)
            nc.scalar.dma_start(wk_sb[:], wkt.rearrange("(kt p) o -> p kt o", p=128))
            wv_sb = consts.tile([128, KT, C], bf16)
            nc.scalar.dma_start(wv_sb[:], wvt.rearrange("(kt p) o -> p kt o", p=128))
            wo_sb = consts.tile([128, KT, C], bf16)
            nc.scalar.dma_start(wo_sb[:], wot.rearrange("(kt p) o -> p kt o", p=128))
            qe_sb = consts.tile([128, XT, 2 * QK], fp8)
            nc.scalar.dma_start(qe_sb[:], qe2.rearrange("(xt p) m -> p xt m", p=128))
            ke_sb = consts.tile([128, XT, 2 * QK], fp8)
            nc.scalar.dma_start(ke_sb[:], ke2.rearrange("(xt p) m -> p xt m", p=128))
            ve_sb = consts.tile([128, X], f32)
            nc.scalar.dma_start(ve_sb[0:QK, :], vet[:])
            nc.scalar.dma_start(ve_sb[QK:128, :], vet[:])

            # vplus double-buffer with the ones columns filled exactly once
            # (they never change; pool rotation would clobber them).
            vplus_bufs = []
            for b in range(2):
                vb = mid.tile([128, KT, 2, X + 2], bf16, tag=f"vplus{b}")
                nc.vector.memset(vb[:, :, :, X:X + 2], 1.0)
                vplus_bufs.append(vb)

            # Software pipeline, 2 pairs deep. The PE queue is FIFO, so the
            # exp -> pu and attn-evac -> o-GEMM waits of pair i are hidden
            # behind pair i+1's projection matmuls:
            #   iter i issues: proj(i), v(i), logits+exp(i), pu(i-1), o(i-2)
            e_hist = {}     # pair -> e_t tile (exp of logits, both wi)
            attn_hist = {}  # pair -> attn tile

            for it in range(PAIRS + 2):
                if it < PAIRS:
                    q_t, k_t, v_t = prefetched if it == 0 else load_inputs(it)

                    # --- q/k projections (fp8 DoubleRow), transposed ---
                    # qsT/khT [x, (h c)] stored fp8 at 16x the projection.
                    qsT = qkt.tile([128, 2, XT, C], fp8)   # [x_p, w, xt, o]
                    khT = qkt.tile([128, 2, XT, C], fp8)
                    for wi in range(2):
                        for xt in range(XT):
                            pq = psA.tile([128, C], f32, tag="mm")
                            for kp in range(KT // 2):
                                nc.tensor.matmul(
                                    pq[:],
                                    q_t[:, 2 * kp:2 * kp + 2, wi,
                                        xt * 128:(xt + 1) * 128],
                                    wq_sb[:, 2 * kp:2 * kp + 2, :],
                                    start=(kp == 0), stop=(kp == KT // 2 - 1),
                                    perf_mode=DR)
                            nc.scalar.activation(qsT[:, wi, xt, :], pq[:],
                                                 AF.Copy, scale=0.25)
                            pk = psA.tile([128, C], f32, tag="mm")
                            for kp in range(KT // 2):
                                nc.tensor.matmul(
                                    pk[:],
                                    k_t[:, 2 * kp:2 * kp + 2, wi,
                                        xt * 128:(xt + 1) * 128],
                                    wk_sb[:, 2 * kp:2 * kp + 2, :],
                                    start=(kp == 0), stop=(kp == KT // 2 - 1),
                                    perf_mode=DR)
                            nc.vector.tensor_scalar_mul(khT[:, wi, xt, :],
                                                        pk[:], 0.25)

                    # --- v projection + ve add (bf16) ---
                    vplus = vplus_bufs[it % 2]   # [c2_p, head-pair, w, x+2]
                    for ot in range(KT):
                        pv = psVL.tile([128, 2, X], f32, tag="vl")
                        for kt in range(KT):
                            nc.tensor.matmul(
                                pv[:],
                                wv_sb[:, kt, ot * 128:(ot + 1) * 128],
                                v_t[:, kt, :, :],
                                start=(kt == 0), stop=(kt == KT - 1))
                        for wi in range(2):
                            nc.vector.tensor_add(
                                vplus[:, ot, wi, 0:X], pv[:, wi, :], ve_sb[:])

                    # --- logits + exp (consumed by pu next iteration) ---
                    e_t = mid.tile([128, 2, C], bf16, tag="e")
                    for wi in range(2):
                        pl = psVL.tile([128, C], f32, tag="vl")
                        # k_emb and q_emb terms, all heads at once (dup'd
                        # tables, fp8 DoubleRow over the two x tiles)
                        nc.tensor.matmul(pl[:], ke_sb[:], khT[:, wi, :, :],
                                         start=True, stop=False, perf_mode=DR)
                        nc.tensor.matmul(pl[:], qe_sb[:], qsT[:, wi, :, :],
                                         start=False, stop=False, perf_mode=DR)
                        # per-head khT^T @ qsT term. The C-half each head's
                        # logits occupy alternates per head-pair (swap for odd
                        # pairs) so the pu matmuls can use all 4 PE quadrants.
                        for h in range(H):
                            half = ((h % 2) ^ ((h // 2) % 2)) * QK
                            cb = h * QK
                            for xt in range(XT):
                                nc.tensor.matmul(
                                    pl[half:half + QK, cb:cb + QK],
                                    khT[:, wi, xt, cb:cb + QK],
                                    qsT[:, wi, xt, cb:cb + QK],
                                    start=False,
                                    stop=(h == H - 1 and xt == XT - 1),
                                    tile_position=(0, half))
                        nc.scalar.activation(e_t[:, wi, :], pl[:], AF.Exp,
                                             scale=1.0 / 4096.0)
                    e_hist[it] = e_t

                if 1 <= it <= PAIRS:
                    # --- attention weights x values for pair it-1 ---
                    pr = it - 1
                    e_t = e_hist.pop(pr)
                    vplus = vplus_bufs[pr % 2]
                    attn = mid.tile([128, KT, 2, X], bf16)  # [(h c)_p, kt, w, x]
                    for wi in range(2):
                        for tp in range(0, KT, 2):   # two head pairs at a time
                            pus = []
                            for t in (tp, tp + 1):
                                pu = psU.tile([128, X + 2], f32, tag="pu")
                                for j in range(2):
                                    h = 2 * t + j
                                    row = ((j ^ (t % 2)) * QK)  # e_t C-half
                                    col = j * QK                # attn layout
                                    nc.tensor.matmul(
                                        pu[col:col + QK, :],
                                        e_t[row:row + QK, wi,
                                            h * QK:(h + 1) * QK],
                                        vplus[row:row + QK, t, wi, :],
                                        start=True, stop=True,
                                        tile_position=(row, col))
                                pus.append(pu)
                            for t, pu in zip((tp, tp + 1), pus):
                                recip = small.tile([128, 1], f32, tag="recip")
                                nc.vector.reciprocal(recip[:], pu[:, X:X + 1])
                                if t != 3:   # scalar is lighter-loaded
                                    nc.scalar.activation(
                                        attn[:, t, wi, :],
                                        pu[:, 0:X], AF.Copy, scale=recip[:])
                                else:
                                    nc.vector.tensor_scalar_mul(
                                        attn[:, t, wi, :], pu[:, 0:X],
                                        recip[:])
                    attn_hist[pr] = attn

                if it >= 2:
                    # --- output projection for pair it-2 ---
                    pr = it - 2
                    w0 = pr * 2
                    attn = attn_hist.pop(pr)
                    for ot in range(KT):
                        po = psA.tile([128, 2, X], f32, tag="mm")
                        for kt in range(KT):
                            nc.tensor.matmul(
                                po[:],
                                wo_sb[:, kt, ot * 128:(ot + 1) * 128],
                                attn[:, kt, :, :],
                                start=(kt == 0), stop=(kt == KT - 1))
                        ob = small.tile([128, 2, X], f32, tag="ob")
                        nc.scalar.activation(ob[:, 0, :], po[:, 0, :], AF.Copy)
                        nc.vector.tensor_copy(ob[:, 1, :], po[:, 1, :])
                        # scalar HWDGE ring: idle after the constant loads, and
                        # keeps output traffic off the input (sync) ring
                        nc.scalar.dma_start(
                            out[ot * 128:(ot + 1) * 128, w0:w0 + 2, :], ob[:])

    nc.compile()
    return nc


def _get_program():
    if "nc" not in _CACHE:
        _CACHE["nc"] = _build_program()
    return _CACHE["nc"]


def _make_in_maps(query, key_, value, Wq, Wk, Wv, Wo, q_emb, k_emb, v_emb):
    import ml_dtypes
    bf16 = ml_dtypes.bfloat16
    fp8 = ml_dtypes.float8_e4m3

    def q8(a, scale):
        return np.ascontiguousarray(
            np.clip(a * np.float32(scale), -240, 240).astype(fp8))

    # Scale plan (logits accumulate 4096x, undone in the exp activation):
    #   qin/kin fp8 at 1x; Wq/Wk fp8 at 64x -> psum 64x; evac scale 0.25
    #   -> qsT/khT fp8 at 16x; per-head term 256x = 4096 * (1/16 softmax).
    #   q_emb fp8 at 16x (pairs with qsT); k_emb fp8 at 256x (pairs with khT,
    #   no softmax scale on the k.ke term).
    wqt = q8(Wq.T, 64.0)
    wkt = q8(Wk.T, 64.0)
    # Swap the two 64-row head blocks inside odd 128-blocks of Wv's output
    # dim: odd head-pairs keep their even/odd heads on swapped partition
    # halves so the pu matmuls can occupy all four PE quadrants.
    vperm = np.arange(C).reshape(C // 128, 2, QK)[:, [0, 1], :].copy()
    vperm[1::2] = vperm[1::2][:, [1, 0], :]
    wvt = np.ascontiguousarray(Wv[vperm.reshape(-1)].T.astype(bf16))
    wot = np.ascontiguousarray(Wo.T.astype(bf16))
    qe2 = q8(np.concatenate([q_emb, q_emb], axis=1), 16.0)
    ke2 = q8(np.concatenate([k_emb, k_emb], axis=1), 256.0)
    vet = np.ascontiguousarray(v_emb.T)

    def shard8(a, ws):
        # (C, X, WC) -> [pair, c, w, x] contiguous fp8
        return q8(
            a[:, :, ws].reshape(C, X, PAIRS, 2).transpose(2, 0, 3, 1), 1.0)

    def shardb(a, ws):
        return np.ascontiguousarray(
            a[:, :, ws].reshape(C, X, PAIRS, 2).transpose(2, 0, 3, 1).astype(bf16))

    in_maps = []
    for c in range(N_CORES):
        ws = slice(c * WC, (c + 1) * WC)
        in_maps.append({
            "qin": shard8(query, ws),
            "kin": shard8(key_, ws),
            "vin": shardb(value, ws),
            "wqt": wqt, "wkt": wkt, "wvt": wvt, "wot": wot,
            "qe2": qe2, "ke2": ke2, "vet": vet,
        })
    return in_maps


def _run(in_maps, trace=False):
    from concourse.bass_utils import run_bass_kernel_spmd
    nc = _get_program()
    return run_bass_kernel_spmd(nc, in_maps, list(range(N_CORES)), trace=trace)


def kernel(query, key_, value, Wq, Wk, Wv, Wo, q_emb, k_emb, v_emb):
    args = (query, key_, value, Wq, Wk, Wv, Wo, q_emb, k_emb, v_emb)
    in_maps = _make_in_maps(*[np.ascontiguousarray(a, np.float32) for a in args])
    res = _run(in_maps, trace=False)
    out = np.empty((C, X, W), np.float32)
    for c in range(N_CORES):
        out[:, :, c * WC:(c + 1) * WC] = res.results[c]["out"].transpose(0, 2, 1)
    return out
